# revision 25
# baseline (speedup 1.0000x reference)
"""Trainium2 Bass kernel for nn_CNN3_P (dense_cnn), 8-core data parallel.

Network (per sample):
  x [128,64] -> pairwise conv -> relu -> [256,127]
  -> conv1d k3 (x3, relu) -> [256,121] -> FC 30976->512 relu -> FC 512->1

Strategy: batch 2048 split 256/core. Channels on partitions (2 chunks of
128); all layers run on a flat [128, T*128] layout (stride 128 per
sample) where the K=3 conv shifts are plain column offsets; boundary
columns hold garbage that never reaches valid outputs. All matmuls in
fp16 (1 cyc/row on the PE; fp32 operands stream at half rate), PSUM
accumulates fp32. Conv3 output is stored (l, s)-major so FC1's
stationary operands are contiguous; Wf1 streams through SBUF once.
"""
import os
import sys

for _p in ('/opt/trn_rl_repo', '/root/.axon_site/_ro/trn_rl_repo'):
    if os.path.isdir(_p) and _p not in sys.path:
        sys.path.insert(0, _p)

import numpy as np
import ml_dtypes

import concourse.bacc as bacc
import concourse.mybir as mybir
import concourse.tile as tile
from concourse.bass_utils import run_bass_kernel_spmd
from concourse.masks import make_identity

F32 = mybir.dt.float32
F16 = mybir.dt.float16

P = 128
CL = 128          # context length
IL = 64           # inst length
PC = 256          # channels (all layers)
NCHUNK = 2        # channel chunks of 128
LF = 121          # conv3 valid positions
F1 = 512
N_CORES = 8
B = 2048
BCORE = B // N_CORES      # 256
T = 8                     # samples per conv sub-tile
NT = BCORE // T           # 32
FLAT = T * CL             # 1024
TILE_N = 512              # psum tile width (4 samples * 128)
NTC = FLAT // TILE_N      # 2
SPT = TILE_N // CL        # samples per psum tile (4)
SC = BCORE // P           # 2 sample chunks of 128 for FC


def build_nc():
    nc = bacc.Bacc("TRN2", target_bir_lowering=False, debug=False)

    xt_d = nc.dram_tensor("xth", [BCORE, IL, CL], F16, kind="ExternalInput")
    xb_d = nc.dram_tensor("xbh", [BCORE, IL, CL], F16, kind="ExternalInput")
    wpc_d = nc.dram_tensor("wpc", [P, PC], F16, kind="ExternalInput")
    bp_d = nc.dram_tensor("bpc", [NCHUNK, P], F32, kind="ExternalInput")
    wc_d = [nc.dram_tensor(f"w{i}t", [NCHUNK, 3, NCHUNK, P, P], F16,
                           kind="ExternalInput") for i in (1, 2, 3)]
    bc_d = [nc.dram_tensor(f"b{i}c", [NCHUNK, P], F32, kind="ExternalInput")
            for i in (1, 2, 3)]
    wf1_d = nc.dram_tensor("wf1t", [NCHUNK, LF, P, F1], F16, kind="ExternalInput")
    bf1_d = nc.dram_tensor("bf1r", [1, F1], F16, kind="ExternalInput")
    ones_d = nc.dram_tensor("onesr", [1, P], F16, kind="ExternalInput")
    wf2_d = nc.dram_tensor("wf2p", [4, P, P], F16, kind="ExternalInput")
    bf2_d = nc.dram_tensor("bf2s", [1, 1], F32, kind="ExternalInput")
    y_d = nc.dram_tensor("y", [BCORE, 1], F32, kind="ExternalOutput")

    RELU = mybir.ActivationFunctionType.Relu

    with tile.TileContext(nc) as tc:
        with tc.tile_pool(name="const", bufs=1) as cpool, \
             tc.tile_pool(name="h3c", bufs=1) as h3pool:
            # --- constants / weights, resident all kernel ---
            wpc = cpool.tile([P, PC], F16)
            nc.sync.dma_start(wpc[:], wpc_d.ap())
            bp = cpool.tile([P, NCHUNK], F32)
            nc.sync.dma_start(bp[:], bp_d.ap().rearrange("c p -> p c"))
            # conv weights: per layer, per ci-chunk: [ci, (k, coc, co)]
            wconv = []
            for i in range(3):
                tiles = []
                for cic in range(NCHUNK):
                    w = cpool.tile([P, 3 * NCHUNK * P], F16, tag=f"w{i}_{cic}")
                    nc.sync.dma_start(
                        w[:].rearrange("p (k b c) -> p k b c", k=3, b=NCHUNK),
                        wc_d[i].ap()[cic].rearrange("k b p c -> p k b c"))
                    tiles.append(w)
                wconv.append(tiles)
            bconv = []
            for i in range(3):
                bt = cpool.tile([P, NCHUNK], F32, tag=f"bc{i}")
                nc.sync.dma_start(bt[:], bc_d[i].ap().rearrange("c p -> p c"))
                bconv.append(bt)

            # persistent conv3 output, fp16, (s, l)-major: col = s*CL + l
            h3c = [h3pool.tile([P, BCORE * CL], F16, tag=f"h3c{cc}", name=f"h3c{cc}")
                   for cc in range(NCHUNK)]
            h3v = [h.rearrange("p (s l) -> p s l", l=CL) for h in h3c]

            # ---------------- conv phase ----------------
            with tc.tile_pool(name="xt", bufs=2) as xtpool, \
                 tc.tile_pool(name="h", bufs=2) as hpool, \
                 tc.tile_pool(name="ps", bufs=8, space="PSUM") as pspool:
                xtv_d = xt_d.ap().rearrange("b j i -> j b i")
                xbv_d = xb_d.ap().rearrange("b j i -> j b i")
                NTS = list(range(NTC - 1, -1, -1))   # nt=1 first: its consumers
                # don't cross the nt boundary, so they unblock earliest

                def pairwise(t):
                    xt = xtpool.tile([P, T * CL], F16, tag="xt", name="xt")
                    nc.gpsimd.dma_start(
                        xt[0:IL, :].rearrange("p (s i) -> p s i", i=CL),
                        xtv_d[:, t * T:(t + 1) * T, :])
                    nc.gpsimd.dma_start(
                        xt[IL:P, :].rearrange("p (s i) -> p s i", i=CL),
                        xbv_d[:, t * T:(t + 1) * T, :])
                    h0 = [hpool.tile([P, FLAT], F16, tag=f"h0_{cc}", name=f"h0_{cc}")
                          for cc in range(NCHUNK)]
                    for nt in NTS:
                        for cc in range(NCHUNK):
                            ps = pspool.tile([P, TILE_N], F32, tag="ps", name="pwps")
                            sl_ = slice(nt * TILE_N, (nt + 1) * TILE_N)
                            nc.tensor.matmul(ps[:], wpc[:, cc * P:(cc + 1) * P],
                                             xt[:, sl_], start=True, stop=True)
                            nc.scalar.activation(h0[cc][:, sl_], ps[:],
                                                 RELU, bias=bp[:, cc:cc + 1])
                    return h0

                def conv_layer(hin, w_tiles, evac):
                    # group-outer: each psum group completes early so its
                    # evacuation overlaps the remaining groups' matmuls
                    for nt in NTS:
                        for co in range(NCHUNK):
                            ps = pspool.tile([P, TILE_N], F32,
                                             tag="ps", name=f"cps{co}_{nt}")
                            step = 0
                            for k in range(3):
                                for ci in range(NCHUNK):
                                    lhsT = w_tiles[ci][:, (k * NCHUNK + co) * P:
                                                       (k * NCHUNK + co + 1) * P]
                                    nk = min(TILE_N, FLAT - nt * TILE_N - k)
                                    nc.tensor.matmul(
                                        ps[:, 0:nk], lhsT,
                                        hin[ci][:, nt * TILE_N + k:
                                                nt * TILE_N + k + nk],
                                        start=(step == 0), stop=(step == 5))
                                    step += 1
                            evac(co, nt, ps)

                h0_next = pairwise(0)
                for t in range(NT):
                    h0 = h0_next
                    h1 = [hpool.tile([P, FLAT], F16, tag=f"h1_{cc}", name=f"h1_{cc}")
                          for cc in range(NCHUNK)]

                    def evac1(co, nt, ps):
                        nc.vector.tensor_scalar(
                            h1[co][:, nt * TILE_N:(nt + 1) * TILE_N], ps[:],
                            bconv[0][:, co:co + 1], 0.0,
                            mybir.AluOpType.add, mybir.AluOpType.max)
                    conv_layer(h0, wconv[0], evac1)

                    # emit next tile's pairwise here so its evacuations age
                    # a full tile before conv1(t+1) consumes them
                    if t + 1 < NT:
                        h0_next = pairwise(t + 1)

                    h2 = [hpool.tile([P, FLAT], F16, tag=f"h2_{cc}", name=f"h2_{cc}")
                          for cc in range(NCHUNK)]

                    def evac2(co, nt, ps):
                        nc.vector.tensor_scalar(
                            h2[co][:, nt * TILE_N:(nt + 1) * TILE_N], ps[:],
                            bconv[1][:, co:co + 1], 0.0,
                            mybir.AluOpType.add, mybir.AluOpType.max)
                    conv_layer(h1, wconv[1], evac2)

                    def evac3(co, nt, ps, t=t):
                        s0 = t * T + nt * SPT
                        nc.scalar.activation(h3c[co][:, s0 * CL:s0 * CL + TILE_N],
                                             ps[:], RELU, bias=bconv[2][:, co:co + 1])
                    conv_layer(h2, wconv[2], evac3)

            # ---------------- FC phase ----------------
            with tc.tile_pool(name="wf1", bufs=3) as wfpool, \
                 tc.tile_pool(name="h4", bufs=1) as h4pool, \
                 tc.tile_pool(name="fps", bufs=2, space="PSUM") as fpspool:
                ident = h4pool.tile([P, P], F16, tag="ident")
                make_identity(nc, ident[:])
                bf1 = h4pool.tile([1, F1], F16, tag="bf1")
                nc.sync.dma_start(bf1[:], bf1_d.ap())
                ones = h4pool.tile([1, P], F16, tag="ones")
                nc.sync.dma_start(ones[:], ones_d.ap())
                wf2 = h4pool.tile([P, 4 * P], F16, tag="wf2")
                nc.sync.dma_start(wf2[:].rearrange("p (f m) -> p f m", f=4),
                                  wf2_d.ap().rearrange("f p m -> p f m"))
                bf2 = h4pool.tile([1, 1], F32, tag="bf2")
                nc.sync.dma_start(bf2[:], bf2_d.ap())
                ps_fc1 = [fpspool.tile([P, F1], F32, tag=f"fc1ps{sc}", bufs=1,
                                       name=f"fc1ps{sc}") for sc in range(SC)]
                for sc in range(SC):
                    nc.tensor.matmul(ps_fc1[sc][:], ones[:], bf1[:],
                                     start=True, stop=False)
                GL = 11   # l-slices per Wf1 DMA (121 = 11*11)
                for cc in range(NCHUNK):
                    for lg in range(LF // GL):
                        rw = wfpool.tile([P, GL * F1], F16, tag="wf1")
                        nc.sync.dma_start(
                            rw[:].rearrange("p (l f) -> p l f", l=GL),
                            wf1_d.ap()[cc, lg * GL:(lg + 1) * GL].rearrange(
                                "l c f -> c l f"))
                        for ll in range(GL):
                            l = lg * GL + ll
                            last = (cc == NCHUNK - 1) and (l == LF - 1)
                            for sc in range(SC):
                                # valid conv3 position l sits at flat l+1
                                nc.tensor.matmul(
                                    ps_fc1[sc][:],
                                    h3v[cc][:, sc * P:(sc + 1) * P, l + 1],
                                    rw[:, ll * F1:(ll + 1) * F1],
                                    start=False, stop=last)
                h4 = []
                for sc in range(SC):
                    h = h4pool.tile([P, F1], F16, tag=f"h4_{sc}", name=f"h4_{sc}")
                    nc.scalar.activation(h[:], ps_fc1[sc][:], RELU)
                    h4.append(h)
                # FC2: transpose h4 then contract f on partitions
                ystage = h4pool.tile([1, BCORE], F32, tag="ystage")
                for sc in range(SC):
                    h4t = h4pool.tile([P, 4 * P], F16, tag=f"h4t_{sc}",
                                      name=f"h4t_{sc}")
                    for fc in range(4):
                        tp = fpspool.tile([P, P], F16, tag="fc2tp", bufs=2)
                        nc.tensor.transpose(tp[:], h4[sc][:, fc * P:(fc + 1) * P],
                                            ident[:])
                        nc.vector.tensor_copy(h4t[:, fc * P:(fc + 1) * P], tp[:])
                    po = fpspool.tile([P, P], F32, tag="fc2ps", bufs=1)
                    for fc in range(4):
                        nc.tensor.matmul(po[:], wf2[:, fc * P:(fc + 1) * P],
                                         h4t[:, fc * P:(fc + 1) * P],
                                         start=(fc == 0), stop=(fc == 3))
                    nc.vector.tensor_scalar_add(ystage[:, sc * P:(sc + 1) * P],
                                                po[0:1, :], bf2[:])
                nc.sync.dma_start(y_d.ap().rearrange("b one -> one b"), ystage[:])

    nc.compile()
    return nc


_NC_CACHE = None


def _get_nc():
    global _NC_CACHE
    if _NC_CACHE is None:
        _NC_CACHE = build_nc()
    return _NC_CACHE


def prep_inputs(x, Wp, bp, W1, b1, W2, b2, W3, b3, Wf1, bf1, Wf2, bf2):
    """Host-side shard + weight re-layout. Returns per-core input maps."""
    f32, f16 = np.float32, np.float16
    wp = np.asarray(Wp, f32)
    wpc = np.ascontiguousarray(
        np.concatenate([wp[:, :, 1].T, wp[:, :, 0].T], axis=0)).astype(f16)
    bpc = np.ascontiguousarray(np.asarray(bp, f32).reshape(NCHUNK, P))

    def conv_t(W):
        # W [co, ci, k] -> [cic, k, coc, ci, co]
        a = np.asarray(W, f32).reshape(NCHUNK, P, NCHUNK, P, 3)
        return np.ascontiguousarray(a.transpose(2, 4, 0, 3, 1)).astype(f16)

    w1t, w2t, w3t = conv_t(W1), conv_t(W2), conv_t(W3)
    b1c = np.ascontiguousarray(np.asarray(b1, f32).reshape(NCHUNK, P))
    b2c = np.ascontiguousarray(np.asarray(b2, f32).reshape(NCHUNK, P))
    b3c = np.ascontiguousarray(np.asarray(b3, f32).reshape(NCHUNK, P))
    # Wf1 [512, 30976] -> [cc, l, c, f] fp16 (contiguous 128KB per (cc, l))
    wf1t = np.ascontiguousarray(
        np.asarray(Wf1, f32).reshape(F1, NCHUNK, P, LF)
        .transpose(1, 3, 2, 0)).astype(f16)
    bf1r = np.ascontiguousarray(np.asarray(bf1, f32).reshape(1, F1)).astype(f16)
    onesr = np.ones((1, P), f16)
    wf2p = np.zeros((4, P, P), f16)
    wf2p[:, :, 0] = np.asarray(Wf2, f32).reshape(4, P)
    bf2s = np.asarray(bf2, f32).reshape(1, 1)

    shared = dict(wpc=wpc, bpc=bpc, w1t=w1t, w2t=w2t, w3t=w3t,
                  b1c=b1c, b2c=b2c, b3c=b3c, wf1t=wf1t, bf1r=bf1r,
                  wf2p=wf2p, bf2s=bf2s, onesr=onesr)
    xr = np.asarray(x, f32).reshape(N_CORES, BCORE, CL, IL).astype(f16)
    xth = np.ascontiguousarray(xr.transpose(0, 1, 3, 2))            # [nc, b, j, i]
    xbh = np.ascontiguousarray(
        np.broadcast_to(xth[:, :, :, 0:1], xth.shape))              # x0 repl over i
    return [dict(xth=xth[i], xbh=xbh[i], **shared) for i in range(N_CORES)]


def kernel(x, Wp, bp, W1, b1, W2, b2, W3, b3, Wf1, bf1, Wf2, bf2,
           trace=False, **run_kwargs):
    nc = _get_nc()
    in_maps = prep_inputs(x, Wp, bp, W1, b1, W2, b2, W3, b3, Wf1, bf1, Wf2, bf2)
    res = run_bass_kernel_spmd(nc, in_maps, core_ids=list(range(N_CORES)),
                               trace=trace, **run_kwargs)
    out = np.concatenate([res.results[i]["y"] for i in range(N_CORES)], axis=0)
    kernel.last_results = res
    return out.astype(np.float32)


kernel.last_results = None
